# revision 22
# baseline (speedup 1.0000x reference)
"""Trainium2 Bass kernel for MiniMHCLM.

Math (HC=4, C=512, K=HC*C=2048, VOCAB=32000, tokens N=B*S=4096):
  x = embed[ids]                               [N, K]
  invr = rsqrt(mean(x^2, -1) + eps)
  mix = (x @ phi) * invr                       [N, 24]
  h_pre  = sigmoid(mix[:, :4]*a_pre + b[:4]) + 0.01
  h_post = sigmoid(mix[:, 4:8]*a_post + b[4:8]) * 2
  h_res  = sinkhorn(mix[:, 8:24]*a_res + b[8:24], 8 iters)  [N,4,4]
  x_in  = sum_i h_pre[i] * x[:, i*C:(i+1)*C]
  f_out = x_in @ W_inner.T
  x_out[o] = sum_i h_res[o,i]*x[i] + h_post[o]*f_out
  logits = x_out.reshape(N, K) @ W_head.T      [N, VOCAB]

Distribution: all 8 cores run the identical coeff path over all tokens;
the head projection is column-sharded over vocab (4000 per core).

Design (v2):
- All PE matmuls in bf16 (1 cyc/row at any moving width, fp32 PSUM
  accumulation).  CPU sim of the quantization chain: ~4e-3 max-rel-err
  vs the 2e-2 gate.
- x^T for the mix matmul comes from the DMA XBAR transpose (2-byte
  dtypes, no PE cost, natural chunk order k = ko*128+p).
- The hyper-channel merge runs on the PE with diag(h) matrices as the
  MOVING operand (4 output channels concatenated -> 512-wide) and
  natural x chunks as stationary; each PSUM chunk is directly a slice
  of x_merge^T, the head's stationary operand.  x_in^T is built the
  same way from diag(h_pre) blocks, so nothing but the indirect gathers
  runs on GPSIMD (whose tensor ops are ~8us each - measured).
- Coefficient nonlinearities (RMS-invr, sigmoid, exp, sinkhorn) are
  batched over 4 tiles: DVE tensor_tensor ops cost ~1.3us nearly
  independent of size, so per-op batching is everything.  All 24
  coefficient scalars per token live in one h_all tile (h_res in (i,o)
  block order, then h_post, h_pre); the sinkhorn normalizes strided
  views of it in place, and ONE tensor_tensor builds the whole
  [P, 24, 128] diag-block tile per token tile.
- Head (vt, tile) matmul chains are interleaved into the per-tile slots
  with paced release so the PE stays dense; W streamed double-buffered
  per vt block.
"""

import sys

for _p in ("/opt/trn_rl_repo", "/root/.axon_site/_ro/trn_rl_repo"):
    if _p not in sys.path:
        sys.path.insert(0, _p)

import ml_dtypes
import numpy as np

import concourse.bass as bass
import concourse.mybir as mybir
import concourse.tile as tile
from concourse.bass_utils import run_bass_kernel_spmd

F32 = mybir.dt.float32
BF16 = mybir.dt.bfloat16
ALU = mybir.AluOpType
ACTF = mybir.ActivationFunctionType
AX = mybir.AxisListType

P = 128
HC, C = 4, 512
K = HC * C  # 2048
KS = K // P  # 16
M = HC * HC + 2 * HC  # 24
RMS_EPS = 1e-6
PRE_EPS = 0.01
TMAX = 8
N_CORES = 8
GROUP_TILES = 8  # tiles per W-block (8 head pairs per vt block)
BATCH = 4  # coefficient batching factor


def legalize_multiwait(nc):
    """Split instructions carrying >1 semaphore wait.

    The walrus build in this image rejects instructions with more than
    one sem wait ("Too many sync wait commands"); Tile emits them
    freely. Move all but the last wait onto standalone InstEventSemaphore
    instructions inserted just before, on the same engine.
    """
    n_fixed = 0
    for fn in nc.m.functions:
        for blk in fn.blocks:
            new = []
            for ins in blk.instructions:
                si = ins.sync_info
                if si is not None and si.on_wait and len(si.on_wait) > 1:
                    waits = list(si.on_wait)
                    for j, w in enumerate(waits[:-1]):
                        es = mybir.InstEventSemaphore(
                            name=f"{ins.name}-w{j}",
                            ins=[],
                            outs=[],
                            sync_info=mybir.SyncInfo(on_wait=[w], on_update=[]),
                        )
                        es.engine = ins.engine
                        nc.register_instruction(es)
                        new.append(es)
                        n_fixed += 1
                    ins.sync_info = mybir.SyncInfo(
                        on_wait=[waits[-1]], on_update=list(si.on_update)
                    )
                new.append(ins)
            blk.instructions[:] = new
    return n_fixed


def build_nc(n_tok, vsh, embed_rows, vt_size=500):
    n_tiles = n_tok // P
    n_vt = vsh // vt_size
    n_groups = n_tiles // GROUP_TILES

    nc = bass.Bass()

    ids_d = nc.dram_tensor("ids", [P, n_tiles], mybir.dt.int32, kind="ExternalInput")
    embed_d = nc.dram_tensor("embed", [embed_rows, K], BF16, kind="ExternalInput")
    wht_d = nc.dram_tensor("wht", [K, vsh], BF16, kind="ExternalInput")
    winner_d = nc.dram_tensor("winner", [C, C], BF16, kind="ExternalInput")
    phi_d = nc.dram_tensor("phi", [K, M], BF16, kind="ExternalInput")
    params_d = nc.dram_tensor("params", [P, 28], F32, kind="ExternalInput")
    ident_d = nc.dram_tensor("ident", [P, P], F32, kind="ExternalInput")
    out_d = nc.dram_tensor("out", [n_tok, vsh], F32, kind="ExternalOutput")

    wht_v = wht_d[:].rearrange("(ko p) v -> p ko v", p=P)  # [128, 16, vsh]

    with tile.TileContext(nc) as tc:
        with (
            tc.tile_pool(name="const", bufs=1) as cpool,
            tc.tile_pool(name="xg", bufs=9) as xg,
            tc.tile_pool(name="xt", bufs=4) as xtp,
            tc.tile_pool(name="sm", bufs=2) as sm,
            tc.tile_pool(name="scr", bufs=2) as scr,
            tc.tile_pool(name="bt", bufs=2) as btp,
            tc.tile_pool(name="dg", bufs=2) as dgp_pool,
            tc.tile_pool(name="xit", bufs=2) as xitp,
            tc.tile_pool(name="fo", bufs=2) as fop,
            tc.tile_pool(name="xmt", bufs=2 * GROUP_TILES) as xmtp,
            tc.tile_pool(name="wp", bufs=2) as wp,
            tc.tile_pool(name="ost", bufs=4) as ostp,
            tc.tile_pool(name="ps_acc", bufs=3, space="PSUM") as ps_acc,
            tc.tile_pool(name="ps_mix", bufs=2, space="PSUM") as ps_mix,
            tc.tile_pool(name="ps_head", bufs=3, space="PSUM") as ps_head,
        ):
            # ---- constants ----
            phi_sb = cpool.tile([P, KS, M], BF16)
            nc.sync.dma_start(phi_sb[:], phi_d[:].rearrange("(ko p) m -> p ko m", p=P))
            winner_sb = cpool.tile([P, HC, C], BF16)
            nc.sync.dma_start(
                winner_sb[:], winner_d[:].rearrange("(ko p) c -> p ko c", p=P)
            )
            bvec = cpool.tile([P, 28], F32)
            nc.sync.dma_start(bvec[:], params_d[:])
            ids_sb = cpool.tile([P, n_tiles], mybir.dt.int32)
            nc.sync.dma_start(ids_sb[:], ids_d[:])
            ident_sb = cpool.tile([P, P], F32)
            nc.sync.dma_start(ident_sb[:], ident_d[:])
            ident_bf = cpool.tile([P, P], BF16)
            nc.vector.tensor_copy(out=ident_bf[:], in_=ident_sb[:])

            state = {}
            batches = {}

            def gather(tt):
                x_r = xg.tile([P, K], BF16, tag="x")
                nc.gpsimd.indirect_dma_start(
                    out=x_r[:],
                    out_offset=None,
                    in_=embed_d[:],
                    in_offset=bass.IndirectOffsetOnAxis(
                        ap=ids_sb[:, tt : tt + 1], axis=0
                    ),
                )
                # x^T via the DMA XBAR transpose: xT[p, ko, t] = x[t, ko*128+p]
                xT = xtp.tile([P, KS, P], BF16, tag="xT")
                nc.sync.dma_start_transpose(xT[:], x_r[:])
                state[tt] = {"x": x_r, "xT": xT}

            def stage1a(tt):
                """Per-tile: mix matmul (PE) -> mix4 column; RMS squares
                (ACT) -> ssq_b column."""
                jb, t4 = tt // BATCH, tt % BATCH
                if t4 == 0:
                    batches[jb] = {
                        "ssq_b": btp.tile(
                            [P, BATCH], F32, tag="ssq_b", name="ssq_b"
                        ),
                        "mix4": btp.tile(
                            [P, BATCH, M], F32, tag="mix4", name="mix4"
                        ),
                        "h_all": btp.tile(
                            [P, BATCH, M], F32, tag="h_all", name="h_all"
                        ),
                    }
                bt = batches[jb]
                st = state[tt]
                x_r, xT = st["x"], st["xT"]

                pma = ps_acc.tile([P, C], F32, tag="acc")
                for ks in range(KS):
                    nc.tensor.matmul(
                        pma[:, :M],
                        xT[:, ks, :],
                        phi_sb[:, ks, :],
                        start=(ks == 0),
                        stop=(ks == KS - 1),
                    )
                nc.vector.tensor_copy(out=bt["mix4"][:, t4, :], in_=pma[:, :M])

                ssqc = sm.tile([P, HC], F32, tag="ssqc")
                for q in range(HC):
                    scratch = scr.tile([P, C], F32, tag="scratch")
                    nc.scalar.activation(
                        out=scratch[:],
                        in_=x_r[:, q * C : (q + 1) * C],
                        func=ACTF.Square,
                        accum_out=ssqc[:, q : q + 1],
                    )
                nc.vector.tensor_reduce(
                    out=bt["ssq_b"][:, t4 : t4 + 1], in_=ssqc[:], axis=AX.X,
                    op=ALU.add,
                )

            def batch_coeffs(jb):
                """All nonlinearities for tiles 4j..4j+3 in batched ops.

                h_all layout per token: cols 0:16 = h_res diag scalars in
                (i,o) block order (col i*4+o = h_res[:, o, i]); cols
                16:20 = h_post (sigmoid; x2 folded into winner); cols
                20:24 = h_pre.
                """
                bt = batches[jb]
                ssq_b, mix4, h_all = bt["ssq_b"], bt["mix4"], bt["h_all"]

                rms4 = sm.tile([P, BATCH], F32, tag="rms4")
                nc.scalar.activation(
                    out=rms4[:], in_=ssq_b[:], func=ACTF.Sqrt, scale=1.0 / K,
                    bias=bvec[:, 27:28],
                )
                invr4 = sm.tile([P, BATCH], F32, tag="invr4")
                nc.vector.reciprocal(out=invr4[:], in_=rms4[:])

                # lg = mix * invr * alpha + b   (in place on mix4)
                nc.vector.tensor_tensor(
                    out=mix4[:], in0=mix4[:],
                    in1=invr4[:, :, None].to_broadcast([P, BATCH, M]),
                    op=ALU.mult,
                )
                nc.vector.tensor_scalar(
                    out=mix4[:, :, 0:4], in0=mix4[:, :, 0:4],
                    scalar1=bvec[:, 24:25], scalar2=None, op0=ALU.mult,
                )
                nc.vector.tensor_scalar(
                    out=mix4[:, :, 4:8], in0=mix4[:, :, 4:8],
                    scalar1=bvec[:, 25:26], scalar2=None, op0=ALU.mult,
                )
                nc.vector.tensor_scalar(
                    out=mix4[:, :, 8:24], in0=mix4[:, :, 8:24],
                    scalar1=bvec[:, 26:27], scalar2=None, op0=ALU.mult,
                )
                nc.vector.tensor_tensor(
                    out=mix4[:], in0=mix4[:],
                    in1=bvec[:, None, 0:24].to_broadcast([P, BATCH, M]),
                    op=ALU.add,
                )

                # h_post / h_pre sigmoids, then exp into (i,o) order
                nc.scalar.activation(
                    out=h_all[:, :, 16:20], in_=mix4[:, :, 4:8],
                    func=ACTF.Sigmoid,
                )
                nc.scalar.activation(
                    out=h_all[:, :, 20:24], in_=mix4[:, :, 0:4],
                    func=ACTF.Sigmoid,
                )
                nc.vector.tensor_scalar(
                    out=h_all[:, :, 20:24], in0=h_all[:, :, 20:24],
                    scalar1=PRE_EPS, scalar2=None, op0=ALU.add,
                )
                # res logits come (o,i)-major; store exp as (i,o)
                nc.scalar.activation(
                    out=h_all[:, :, 0:16].rearrange("p g (i o) -> p g i o", o=HC),
                    in_=mix4[:, :, 8:24].rearrange("p g (o i) -> p g i o", i=HC),
                    func=ACTF.Exp,
                )

                # sinkhorn in place on h_all[:, :, 0:16]
                vio = h_all[:, :, 0:16].rearrange("p g (i o) -> p g o i", o=HC)
                voi = h_all[:, :, 0:16].rearrange("p g (i o) -> p g i o", o=HC)
                rsum = sm.tile([P, BATCH, HC], F32, tag="rsum")
                rrec = sm.tile([P, BATCH, HC], F32, tag="rrec")

                def norm(view):
                    nc.vector.tensor_reduce(
                        out=rsum[:], in_=view, axis=AX.X, op=ALU.add
                    )
                    nc.vector.reciprocal(out=rrec[:], in_=rsum[:])
                    nc.vector.tensor_tensor(
                        out=view, in0=view,
                        in1=rrec[:, :, :, None].to_broadcast([P, BATCH, HC, HC]),
                        op=ALU.mult,
                    )

                norm(vio)  # softmax denominator (sum over i at fixed o)
                norm(voi)
                for _ in range(TMAX - 1):
                    norm(vio)
                    norm(voi)

            xmt_tiles = {}

            def stage2(tt):
                """diag tile + x_in^T + f_out + transposed merge -> xmt."""
                jb, t4 = tt // BATCH, tt % BATCH
                h_all = batches[jb]["h_all"]
                st = state[tt]
                x_r = st["x"]

                # ONE op builds all 24 diag blocks for this tile
                dgall = dgp_pool.tile([P, M, P], BF16, tag="dgall")
                h_t = h_all[:, t4 : t4 + 1, :].rearrange("p o m -> p (o m)")
                nc.vector.tensor_tensor(
                    out=dgall[:],
                    in0=ident_bf[:, None, :].to_broadcast([P, M, P]),
                    in1=h_t[:, :, None].to_broadcast([P, M, P]),
                    op=ALU.mult,
                )

                # x_in^T chunks via diag(h_pre) moving blocks
                px = ps_acc.tile([P, C], F32, tag="acc")
                px_v = px[:].rearrange("p (a b) -> p a b", a=HC)
                for cb in range(HC):
                    for i in range(HC):
                        nc.tensor.matmul(
                            px_v[:, cb, :],
                            x_r[:, i * C + cb * P : i * C + (cb + 1) * P],
                            dgall[:, 20 + i, :],
                            start=(i == 0),
                            stop=(i == HC - 1),
                        )
                xiT = xitp.tile([P, HC, P], BF16, tag="xiT")
                nc.scalar.copy(
                    out=xiT[:].rearrange("p a b -> p (a b)"), in_=px[:]
                )

                pf = ps_acc.tile([P, C], F32, tag="acc")
                for cb in range(HC):
                    nc.tensor.matmul(
                        pf[:],
                        xiT[:, cb, :],
                        winner_sb[:, cb, :],
                        start=(cb == 0),
                        stop=(cb == HC - 1),
                    )
                fout = fop.tile([P, C], BF16, tag="fout")
                nc.scalar.copy(out=fout[:], in_=pf[:])

                # transposed merge: pm[c, o*128+t] accumulates
                #   sum_i x[t, i*C+cb*128+c]*h_res[t,o,i] + f[t,...]*h_post[t,o]
                xmt = xmtp.tile([P, KS, P], BF16, tag="xmt")
                xmt_v = xmt[:].rearrange("p (o c) t -> p c o t", c=HC)
                for cb in range(HC):
                    pm = ps_mix.tile([P, HC, P], F32, tag="pm")
                    pm_f = pm[:].rearrange("p a b -> p (a b)")
                    for i in range(HC):
                        nc.tensor.matmul(
                            pm_f,
                            x_r[:, i * C + cb * P : i * C + (cb + 1) * P],
                            dgall[:, i * HC : (i + 1) * HC, :].rearrange(
                                "p a b -> p (a b)"
                            ),
                            start=(i == 0),
                            stop=False,
                        )
                    nc.tensor.matmul(
                        pm_f,
                        fout[:, cb * P : (cb + 1) * P],
                        dgall[:, 16:20, :].rearrange("p a b -> p (a b)"),
                        start=False,
                        stop=True,
                    )
                    nc.scalar.copy(out=xmt_v[:, cb], in_=pm[:])
                xmt_tiles[tt] = xmt
                del state[tt]

            # ---- W fetch management (vt-major blocks of 8 pairs) ----
            n_blocks = n_groups * n_vt
            w_ring = {}
            w_fetched = 0

            def fetch_next_w():
                nonlocal w_fetched
                if w_fetched >= n_blocks:
                    return
                vt = w_fetched % n_vt
                w_sb = wp.tile([P, KS, vt_size], BF16, tag="w")
                for kq in range(4):
                    nc.sync.dma_start(
                        w_sb[:, 4 * kq : 4 * kq + 4, :],
                        wht_v[
                            :, 4 * kq : 4 * kq + 4,
                            vt * vt_size : (vt + 1) * vt_size,
                        ],
                    )
                w_ring[w_fetched] = w_sb
                w_fetched += 1

            def emit_pair(idx):
                """Head chain for pair idx; alternate PSUM-drain engine."""
                blk = idx // GROUP_TILES
                g, vt = blk // n_vt, blk % n_vt
                tt = g * GROUP_TILES + idx % GROUP_TILES
                while w_fetched <= blk + 1 and w_fetched < n_blocks:
                    fetch_next_w()
                w_sb = w_ring[blk]
                ph = ps_head.tile([P, vt_size], F32, tag="ph")
                xmt = xmt_tiles[tt]
                for ks in range(KS):
                    nc.tensor.matmul(
                        ph[:],
                        xmt[:, ks, :],
                        w_sb[:, ks, :],
                        start=(ks == 0),
                        stop=(ks == KS - 1),
                    )
                ost = ostp.tile([P, vt_size], F32, tag="ost")
                if idx % 2 == 0:
                    nc.scalar.copy(out=ost[:], in_=ph[:])
                else:
                    nc.vector.tensor_copy(out=ost[:], in_=ph[:])
                nc.sync.dma_start(
                    out_d[
                        tt * P : (tt + 1) * P,
                        vt * vt_size : (vt + 1) * vt_size,
                    ],
                    ost[:],
                )
                if idx % GROUP_TILES == GROUP_TILES - 1:
                    del w_ring[blk]

            pair_tile = [
                g * GROUP_TILES + t8
                for g in range(n_groups)
                for _vt in range(n_vt)
                for t8 in range(GROUP_TILES)
            ]
            n_pairs = len(pair_tile)

            # ---- main schedule ----
            for tt in range(3):
                gather(tt)
            fetch_next_w()
            fetch_next_w()

            ptr = 0
            for slot in range(n_tiles + BATCH):
                if slot < n_tiles:
                    if slot + 3 < n_tiles:
                        gather(slot + 3)
                    stage1a(slot)
                    if slot % BATCH == BATCH - 1:
                        batch_coeffs(slot // BATCH)
                budget = 10
                while (
                    ptr < n_pairs
                    and budget > 0
                    and pair_tile[ptr] <= slot - (BATCH + 1)
                ):
                    emit_pair(ptr)
                    ptr += 1
                    budget -= 1
                t2 = slot - BATCH
                if 0 <= t2 < n_tiles:
                    stage2(t2)
            while ptr < n_pairs:
                emit_pair(ptr)
                ptr += 1

    legalize_multiwait(nc)
    return nc


LAST_RESULT = None


def kernel(input_ids, embed, W_inner, W_head, phi, b,
           alpha_pre, alpha_post, alpha_res):
    global LAST_RESULT
    ids = np.asarray(input_ids).reshape(-1).astype(np.int32)
    B, S = np.asarray(input_ids).shape
    n_tok = ids.size
    n_tiles = n_tok // P
    embed_bf = np.ascontiguousarray(
        np.asarray(embed, dtype=np.float32).astype(ml_dtypes.bfloat16)
    )
    vocab = embed_bf.shape[0]
    vsh = vocab // N_CORES

    ids_pm = np.ascontiguousarray(ids.reshape(n_tiles, P).T)  # [128, n_tiles]
    wht_full = np.ascontiguousarray(
        np.asarray(W_head, np.float32).T.astype(ml_dtypes.bfloat16)
    )  # [K, vocab]
    winner = np.ascontiguousarray(
        (np.asarray(W_inner, np.float32).T * np.float32(2.0)).astype(
            ml_dtypes.bfloat16
        )
    )
    phi_np = np.ascontiguousarray(
        np.asarray(phi, np.float32).astype(ml_dtypes.bfloat16)
    )
    params = np.zeros((P, 28), np.float32)
    params[:, :24] = np.asarray(b, np.float32)[None, :]
    params[:, 24] = np.float32(alpha_pre)
    params[:, 25] = np.float32(alpha_post)
    params[:, 26] = np.float32(alpha_res)
    params[:, 27] = np.float32(RMS_EPS)
    ident = np.eye(P, dtype=np.float32)

    nc = build_nc(n_tok=n_tok, vsh=vsh, embed_rows=vocab)

    in_maps = []
    for c in range(N_CORES):
        in_maps.append(
            {
                "ids": ids_pm,
                "embed": embed_bf,
                "wht": np.ascontiguousarray(
                    wht_full[:, c * vsh : (c + 1) * vsh]
                ),
                "winner": winner,
                "phi": phi_np,
                "params": params,
                "ident": ident,
            }
        )
    res = run_bass_kernel_spmd(nc, in_maps, core_ids=list(range(N_CORES)))
    LAST_RESULT = res
    logits = np.concatenate(
        [res.results[c]["out"] for c in range(N_CORES)], axis=1
    )
    return logits.reshape(B, S, vocab).astype(np.float32)


# revision 25
# speedup vs baseline: 1.2050x; 1.2050x over previous
"""Trainium2 Bass kernel for MiniMHCLM.

Math (HC=4, C=512, K=HC*C=2048, VOCAB=32000, tokens N=B*S=4096):
  x = embed[ids]                               [N, K]
  invr = rsqrt(mean(x^2, -1) + eps)
  mix = (x @ phi) * invr                       [N, 24]
  h_pre  = sigmoid(mix[:, :4]*a_pre + b[:4]) + 0.01
  h_post = sigmoid(mix[:, 4:8]*a_post + b[4:8]) * 2
  h_res  = sinkhorn(mix[:, 8:24]*a_res + b[8:24], 8 iters)  [N,4,4]
  x_in  = sum_i h_pre[i] * x[:, i*C:(i+1)*C]
  f_out = x_in @ W_inner.T
  x_out[o] = sum_i h_res[o,i]*x[i] + h_post[o]*f_out
  logits = x_out.reshape(N, K) @ W_head.T      [N, VOCAB]

Distribution: all 8 cores run the identical coeff path over all tokens;
the head projection is column-sharded over vocab (4000 per core).

Design (v2):
- All PE matmuls in bf16 (1 cyc/row at any moving width, fp32 PSUM
  accumulation).  CPU sim of the quantization chain: ~4e-3 max-rel-err
  vs the 2e-2 gate.
- x^T for the mix matmul comes from the DMA XBAR transpose (2-byte
  dtypes, no PE cost, natural chunk order k = ko*128+p).
- The hyper-channel merge runs on the PE with diag(h) matrices as the
  MOVING operand (4 output channels concatenated -> 512-wide) and
  natural x chunks as stationary; each PSUM chunk is directly a slice
  of x_merge^T, the head's stationary operand.  x_in^T is built the
  same way from diag(h_pre) blocks, so nothing but the indirect gathers
  runs on GPSIMD (whose tensor ops are ~8us each - measured).
- Coefficient nonlinearities (RMS-invr, sigmoid, exp, sinkhorn) are
  batched over 4 tiles: DVE tensor_tensor ops cost ~1.3us nearly
  independent of size, so per-op batching is everything.  All 24
  coefficient scalars per token live in one h_all tile (h_res in (i,o)
  block order, then h_post, h_pre); the sinkhorn normalizes strided
  views of it in place, and ONE tensor_tensor builds the whole
  [P, 24, 128] diag-block tile per token tile.
- Head (vt, tile) matmul chains are interleaved into the per-tile slots
  with paced release so the PE stays dense; W streamed double-buffered
  per vt block.
"""

import sys

for _p in ("/opt/trn_rl_repo", "/root/.axon_site/_ro/trn_rl_repo"):
    if _p not in sys.path:
        sys.path.insert(0, _p)

import ml_dtypes
import numpy as np

import concourse.bass as bass
import concourse.mybir as mybir
import concourse.tile as tile
from concourse.bass_utils import run_bass_kernel_spmd

F32 = mybir.dt.float32
BF16 = mybir.dt.bfloat16
ALU = mybir.AluOpType
ACTF = mybir.ActivationFunctionType
AX = mybir.AxisListType

P = 128
HC, C = 4, 512
K = HC * C  # 2048
KS = K // P  # 16
M = HC * HC + 2 * HC  # 24
RMS_EPS = 1e-6
PRE_EPS = 0.01
TMAX = 8
N_CORES = 8
GROUP_TILES = 8  # tiles per W-block (8 head pairs per vt block)
BATCH = 4  # coefficient batching factor


def legalize_multiwait(nc):
    """Split instructions carrying >1 semaphore wait.

    The walrus build in this image rejects instructions with more than
    one sem wait ("Too many sync wait commands"); Tile emits them
    freely. Move all but the last wait onto standalone InstEventSemaphore
    instructions inserted just before, on the same engine.
    """
    n_fixed = 0
    for fn in nc.m.functions:
        for blk in fn.blocks:
            new = []
            for ins in blk.instructions:
                si = ins.sync_info
                if si is not None and si.on_wait and len(si.on_wait) > 1:
                    waits = list(si.on_wait)
                    for j, w in enumerate(waits[:-1]):
                        es = mybir.InstEventSemaphore(
                            name=f"{ins.name}-w{j}",
                            ins=[],
                            outs=[],
                            sync_info=mybir.SyncInfo(on_wait=[w], on_update=[]),
                        )
                        es.engine = ins.engine
                        nc.register_instruction(es)
                        new.append(es)
                        n_fixed += 1
                    ins.sync_info = mybir.SyncInfo(
                        on_wait=[waits[-1]], on_update=list(si.on_update)
                    )
                new.append(ins)
            blk.instructions[:] = new
    return n_fixed


def build_nc(n_tok, vsh, embed_rows, vt_size=500):
    n_tiles = n_tok // P
    n_vt = vsh // vt_size
    n_groups = n_tiles // GROUP_TILES

    nc = bass.Bass()

    ids_d = nc.dram_tensor("ids", [P, n_tiles], mybir.dt.int32, kind="ExternalInput")
    embed_d = nc.dram_tensor("embed", [embed_rows, K], BF16, kind="ExternalInput")
    wht_d = nc.dram_tensor("wht", [K, vsh], BF16, kind="ExternalInput")
    winner_d = nc.dram_tensor("winner", [C, C], BF16, kind="ExternalInput")
    phi_d = nc.dram_tensor("phi", [K, M], BF16, kind="ExternalInput")
    params_d = nc.dram_tensor("params", [P, 28], F32, kind="ExternalInput")
    ident_d = nc.dram_tensor("ident", [P, P], F32, kind="ExternalInput")
    out_d = nc.dram_tensor("out", [n_tok, vsh], F32, kind="ExternalOutput")

    wht_v = wht_d[:].rearrange("(ko p) v -> p ko v", p=P)  # [128, 16, vsh]

    with tile.TileContext(nc) as tc:
        with (
            tc.tile_pool(name="const", bufs=1) as cpool,
            tc.tile_pool(name="xg", bufs=9) as xg,
            tc.tile_pool(name="xt", bufs=4) as xtp,
            tc.tile_pool(name="sm", bufs=2) as sm,
            tc.tile_pool(name="scr", bufs=2) as scr,
            tc.tile_pool(name="bt", bufs=2) as btp,
            tc.tile_pool(name="dg", bufs=2) as dgp_pool,
            tc.tile_pool(name="xin", bufs=2) as xinp,
            tc.tile_pool(name="xit", bufs=2) as xitp,
            tc.tile_pool(name="fo", bufs=2) as fop,
            tc.tile_pool(name="xmt", bufs=2 * GROUP_TILES) as xmtp,
            tc.tile_pool(name="wp", bufs=2) as wp,
            tc.tile_pool(name="ost", bufs=4) as ostp,
            tc.tile_pool(name="ps_acc", bufs=2, space="PSUM") as ps_acc,
            tc.tile_pool(name="ps_mix", bufs=2, space="PSUM") as ps_mix,
            tc.tile_pool(name="ps_head", bufs=4, space="PSUM") as ps_head,
        ):
            # ---- constants ----
            phi_sb = cpool.tile([P, KS, M], BF16)
            nc.sync.dma_start(phi_sb[:], phi_d[:].rearrange("(ko p) m -> p ko m", p=P))
            winner_sb = cpool.tile([P, HC, C], BF16)
            nc.sync.dma_start(
                winner_sb[:], winner_d[:].rearrange("(ko p) c -> p ko c", p=P)
            )
            bvec = cpool.tile([P, 28], F32)
            nc.sync.dma_start(bvec[:], params_d[:])
            ids_sb = cpool.tile([P, n_tiles], mybir.dt.int32)
            nc.sync.dma_start(ids_sb[:], ids_d[:])
            ident_sb = cpool.tile([P, P], F32)
            nc.sync.dma_start(ident_sb[:], ident_d[:])
            ident_bf = cpool.tile([P, P], BF16)
            nc.vector.tensor_copy(out=ident_bf[:], in_=ident_sb[:])

            state = {}
            batches = {}

            def gather(tt):
                x_r = xg.tile([P, K], BF16, tag="x")
                nc.gpsimd.indirect_dma_start(
                    out=x_r[:],
                    out_offset=None,
                    in_=embed_d[:],
                    in_offset=bass.IndirectOffsetOnAxis(
                        ap=ids_sb[:, tt : tt + 1], axis=0
                    ),
                )
                # x^T via the DMA XBAR transpose: xT[p, ko, t] = x[t, ko*128+p]
                xT = xtp.tile([P, KS, P], BF16, tag="xT")
                nc.sync.dma_start_transpose(xT[:], x_r[:])
                state[tt] = {"x": x_r, "xT": xT}

            def stage1a(tt):
                """Per-tile: mix matmul (PE) -> mix4 column; RMS squares
                (ACT) -> ssq_b column."""
                jb, t4 = tt // BATCH, tt % BATCH
                if t4 == 0:
                    batches[jb] = {
                        "ssq_b": btp.tile(
                            [P, BATCH], F32, tag="ssq_b", name="ssq_b"
                        ),
                        "mix4": btp.tile(
                            [P, BATCH, M], F32, tag="mix4", name="mix4"
                        ),
                        "h_all": btp.tile(
                            [P, BATCH, M], F32, tag="h_all", name="h_all"
                        ),
                    }
                bt = batches[jb]
                st = state[tt]
                x_r, xT = st["x"], st["xT"]

                pma = ps_acc.tile([P, C], F32, tag="acc")
                for ks in range(KS):
                    nc.tensor.matmul(
                        pma[:, :M],
                        xT[:, ks, :],
                        phi_sb[:, ks, :],
                        start=(ks == 0),
                        stop=(ks == KS - 1),
                    )
                nc.vector.tensor_copy(out=bt["mix4"][:, t4, :], in_=pma[:, :M])

                ssqc = sm.tile([P, HC], F32, tag="ssqc")
                for q in range(HC):
                    scratch = scr.tile([P, C], F32, tag="scratch")
                    nc.scalar.activation(
                        out=scratch[:],
                        in_=x_r[:, q * C : (q + 1) * C],
                        func=ACTF.Square,
                        accum_out=ssqc[:, q : q + 1],
                    )
                nc.vector.tensor_reduce(
                    out=bt["ssq_b"][:, t4 : t4 + 1], in_=ssqc[:], axis=AX.X,
                    op=ALU.add,
                )

            def batch_coeffs(jb):
                """All nonlinearities for tiles 4j..4j+3 in batched ops.

                h_all layout per token: cols 0:16 = h_res diag scalars in
                (i,o) block order (col i*4+o = h_res[:, o, i]); cols
                16:20 = h_post (sigmoid; x2 folded into winner); cols
                20:24 = h_pre.
                """
                bt = batches[jb]
                ssq_b, mix4, h_all = bt["ssq_b"], bt["mix4"], bt["h_all"]

                rms4 = sm.tile([P, BATCH], F32, tag="rms4")
                nc.scalar.activation(
                    out=rms4[:], in_=ssq_b[:], func=ACTF.Sqrt, scale=1.0 / K,
                    bias=bvec[:, 27:28],
                )
                invr4 = sm.tile([P, BATCH], F32, tag="invr4")
                nc.vector.reciprocal(out=invr4[:], in_=rms4[:])

                # lg = mix * invr * alpha + b   (in place on mix4)
                nc.vector.tensor_tensor(
                    out=mix4[:], in0=mix4[:],
                    in1=invr4[:, :, None].to_broadcast([P, BATCH, M]),
                    op=ALU.mult,
                )
                nc.vector.tensor_scalar(
                    out=mix4[:, :, 0:4], in0=mix4[:, :, 0:4],
                    scalar1=bvec[:, 24:25], scalar2=None, op0=ALU.mult,
                )
                nc.vector.tensor_scalar(
                    out=mix4[:, :, 4:8], in0=mix4[:, :, 4:8],
                    scalar1=bvec[:, 25:26], scalar2=None, op0=ALU.mult,
                )
                nc.vector.tensor_scalar(
                    out=mix4[:, :, 8:24], in0=mix4[:, :, 8:24],
                    scalar1=bvec[:, 26:27], scalar2=None, op0=ALU.mult,
                )
                nc.vector.tensor_tensor(
                    out=mix4[:], in0=mix4[:],
                    in1=bvec[:, None, 0:24].to_broadcast([P, BATCH, M]),
                    op=ALU.add,
                )

                # h_post / h_pre sigmoids, then exp into (i,o) order
                nc.scalar.activation(
                    out=h_all[:, :, 16:20], in_=mix4[:, :, 4:8],
                    func=ACTF.Sigmoid,
                )
                nc.scalar.activation(
                    out=h_all[:, :, 20:24], in_=mix4[:, :, 0:4],
                    func=ACTF.Sigmoid,
                )
                nc.vector.tensor_scalar(
                    out=h_all[:, :, 20:24], in0=h_all[:, :, 20:24],
                    scalar1=PRE_EPS, scalar2=None, op0=ALU.add,
                )
                # res logits come (o,i)-major; store exp as (i,o)
                nc.scalar.activation(
                    out=h_all[:, :, 0:16].rearrange("p g (i o) -> p g i o", o=HC),
                    in_=mix4[:, :, 8:24].rearrange("p g (o i) -> p g i o", i=HC),
                    func=ACTF.Exp,
                )

                # sinkhorn in place on h_all[:, :, 0:16]
                vio = h_all[:, :, 0:16].rearrange("p g (i o) -> p g o i", o=HC)
                voi = h_all[:, :, 0:16].rearrange("p g (i o) -> p g i o", o=HC)
                rsum = sm.tile([P, BATCH, HC], F32, tag="rsum")
                rrec = sm.tile([P, BATCH, HC], F32, tag="rrec")

                def norm(view):
                    nc.vector.tensor_reduce(
                        out=rsum[:], in_=view, axis=AX.X, op=ALU.add
                    )
                    nc.vector.reciprocal(out=rrec[:], in_=rsum[:])
                    nc.vector.tensor_tensor(
                        out=view, in0=view,
                        in1=rrec[:, :, :, None].to_broadcast([P, BATCH, HC, HC]),
                        op=ALU.mult,
                    )

                norm(vio)  # softmax denominator (sum over i at fixed o)
                norm(voi)
                for _ in range(TMAX - 1):
                    norm(vio)
                    norm(voi)

            xmt_tiles = {}

            def stage2(tt):
                """diag tile + x_in + f_out + transposed merge -> xmt."""
                jb, t4 = tt // BATCH, tt % BATCH
                h_all = batches[jb]["h_all"]
                st = state[tt]
                x_r = st["x"]

                # x_in = sum_i h_pre[i]*x_i  (DVE: 4 scaled chunks + reduce)
                xstg = xinp.tile([P, HC, C], BF16, tag="xstg")
                for i in range(HC):
                    hp = h_all[
                        :, t4 : t4 + 1, 20 + i : 21 + i
                    ].rearrange("p a b -> p (a b)")
                    nc.vector.tensor_scalar(
                        out=xstg[:, i, :], in0=x_r[:, i * C : (i + 1) * C],
                        scalar1=hp, scalar2=None, op0=ALU.mult,
                    )
                xi = xinp.tile([P, C], BF16, tag="xi")
                with nc.allow_low_precision(reason="4-term bf16 x_in sum"):
                    nc.vector.tensor_reduce(
                        out=xi[:], in_=xstg[:].rearrange("p i c -> p c i"),
                        axis=AX.X, op=ALU.add,
                    )
                # x_in^T via XBAR transpose: xiT[p, cb, t] = x_in[t, cb*128+p]
                xiT = xitp.tile([P, HC, P], BF16, tag="xiT")
                nc.sync.dma_start_transpose(xiT[:], xi[:])

                # ONE op builds the 20 diag blocks (16 h_res + 4 h_post)
                dgall = dgp_pool.tile([P, 20, P], BF16, tag="dgall")
                h_t = h_all[:, t4 : t4 + 1, 0:20].rearrange("p o m -> p (o m)")
                nc.vector.tensor_tensor(
                    out=dgall[:],
                    in0=ident_bf[:, None, :].to_broadcast([P, 20, P]),
                    in1=h_t[:, :, None].to_broadcast([P, 20, P]),
                    op=ALU.mult,
                )

                pf = ps_acc.tile([P, C], F32, tag="acc")
                for cb in range(HC):
                    nc.tensor.matmul(
                        pf[:],
                        xiT[:, cb, :],
                        winner_sb[:, cb, :],
                        start=(cb == 0),
                        stop=(cb == HC - 1),
                    )
                fout = fop.tile([P, C], BF16, tag="fout")
                nc.scalar.copy(out=fout[:], in_=pf[:])

                # transposed merge: pm[c, o*128+t] accumulates
                #   sum_i x[t, i*C+cb*128+c]*h_res[t,o,i] + f[t,...]*h_post[t,o]
                xmt = xmtp.tile([P, KS, P], BF16, tag="xmt")
                xmt_v = xmt[:].rearrange("p (o c) t -> p c o t", c=HC)
                for cb in range(HC):
                    pm = ps_mix.tile([P, HC, P], F32, tag="pm")
                    pm_f = pm[:].rearrange("p a b -> p (a b)")
                    for i in range(HC):
                        nc.tensor.matmul(
                            pm_f,
                            x_r[:, i * C + cb * P : i * C + (cb + 1) * P],
                            dgall[:, i * HC : (i + 1) * HC, :].rearrange(
                                "p a b -> p (a b)"
                            ),
                            start=(i == 0),
                            stop=False,
                        )
                    nc.tensor.matmul(
                        pm_f,
                        fout[:, cb * P : (cb + 1) * P],
                        dgall[:, 16:20, :].rearrange("p a b -> p (a b)"),
                        start=False,
                        stop=True,
                    )
                    nc.scalar.copy(out=xmt_v[:, cb], in_=pm[:])
                xmt_tiles[tt] = xmt
                del state[tt]

            # ---- W fetch management (vt-major blocks of 8 pairs) ----
            n_blocks = n_groups * n_vt
            w_ring = {}
            w_fetched = 0

            def fetch_next_w():
                nonlocal w_fetched
                if w_fetched >= n_blocks:
                    return
                vt = w_fetched % n_vt
                w_sb = wp.tile([P, KS, vt_size], BF16, tag="w")
                for kq in range(4):
                    nc.sync.dma_start(
                        w_sb[:, 4 * kq : 4 * kq + 4, :],
                        wht_v[
                            :, 4 * kq : 4 * kq + 4,
                            vt * vt_size : (vt + 1) * vt_size,
                        ],
                    )
                w_ring[w_fetched] = w_sb
                w_fetched += 1

            def emit_pair(idx):
                """Head chain for pair idx; alternate PSUM-drain engine."""
                blk = idx // GROUP_TILES
                g, vt = blk // n_vt, blk % n_vt
                tt = g * GROUP_TILES + idx % GROUP_TILES
                while w_fetched <= blk + 1 and w_fetched < n_blocks:
                    fetch_next_w()
                w_sb = w_ring[blk]
                ph = ps_head.tile([P, vt_size], F32, tag="ph")
                xmt = xmt_tiles[tt]
                for ks in range(KS):
                    nc.tensor.matmul(
                        ph[:],
                        xmt[:, ks, :],
                        w_sb[:, ks, :],
                        start=(ks == 0),
                        stop=(ks == KS - 1),
                    )
                ost = ostp.tile([P, vt_size], F32, tag="ost")
                if idx % 2 == 0:
                    nc.scalar.copy(out=ost[:], in_=ph[:])
                else:
                    nc.vector.tensor_copy(out=ost[:], in_=ph[:])
                nc.sync.dma_start(
                    out_d[
                        tt * P : (tt + 1) * P,
                        vt * vt_size : (vt + 1) * vt_size,
                    ],
                    ost[:],
                )
                if idx % GROUP_TILES == GROUP_TILES - 1:
                    del w_ring[blk]

            pair_tile = [
                g * GROUP_TILES + t8
                for g in range(n_groups)
                for _vt in range(n_vt)
                for t8 in range(GROUP_TILES)
            ]
            n_pairs = len(pair_tile)

            # ---- main schedule ----
            for tt in range(3):
                gather(tt)
            fetch_next_w()
            fetch_next_w()

            ptr = 0
            for slot in range(n_tiles + BATCH):
                if slot < n_tiles:
                    if slot + 3 < n_tiles:
                        gather(slot + 3)
                    stage1a(slot)
                    if slot % BATCH == BATCH - 1:
                        batch_coeffs(slot // BATCH)
                budget = 10
                while (
                    ptr < n_pairs
                    and budget > 0
                    and pair_tile[ptr] <= slot - (BATCH + 1)
                ):
                    emit_pair(ptr)
                    ptr += 1
                    budget -= 1
                t2 = slot - BATCH
                if 0 <= t2 < n_tiles:
                    stage2(t2)
            while ptr < n_pairs:
                emit_pair(ptr)
                ptr += 1

    legalize_multiwait(nc)
    return nc


LAST_RESULT = None


def kernel(input_ids, embed, W_inner, W_head, phi, b,
           alpha_pre, alpha_post, alpha_res):
    global LAST_RESULT
    ids = np.asarray(input_ids).reshape(-1).astype(np.int32)
    B, S = np.asarray(input_ids).shape
    n_tok = ids.size
    n_tiles = n_tok // P
    embed_bf = np.ascontiguousarray(
        np.asarray(embed, dtype=np.float32).astype(ml_dtypes.bfloat16)
    )
    vocab = embed_bf.shape[0]
    vsh = vocab // N_CORES

    ids_pm = np.ascontiguousarray(ids.reshape(n_tiles, P).T)  # [128, n_tiles]
    wht_full = np.ascontiguousarray(
        np.asarray(W_head, np.float32).T.astype(ml_dtypes.bfloat16)
    )  # [K, vocab]
    winner = np.ascontiguousarray(
        (np.asarray(W_inner, np.float32).T * np.float32(2.0)).astype(
            ml_dtypes.bfloat16
        )
    )
    phi_np = np.ascontiguousarray(
        np.asarray(phi, np.float32).astype(ml_dtypes.bfloat16)
    )
    params = np.zeros((P, 28), np.float32)
    params[:, :24] = np.asarray(b, np.float32)[None, :]
    params[:, 24] = np.float32(alpha_pre)
    params[:, 25] = np.float32(alpha_post)
    params[:, 26] = np.float32(alpha_res)
    params[:, 27] = np.float32(RMS_EPS)
    ident = np.eye(P, dtype=np.float32)

    nc = build_nc(n_tok=n_tok, vsh=vsh, embed_rows=vocab)

    in_maps = []
    for c in range(N_CORES):
        in_maps.append(
            {
                "ids": ids_pm,
                "embed": embed_bf,
                "wht": np.ascontiguousarray(
                    wht_full[:, c * vsh : (c + 1) * vsh]
                ),
                "winner": winner,
                "phi": phi_np,
                "params": params,
                "ident": ident,
            }
        )
    res = run_bass_kernel_spmd(nc, in_maps, core_ids=list(range(N_CORES)))
    LAST_RESULT = res
    logits = np.concatenate(
        [res.results[c]["out"] for c in range(N_CORES)], axis=1
    )
    return logits.reshape(B, S, vocab).astype(np.float32)
